# revision 26
# baseline (speedup 1.0000x reference)
"""Causal self-attention (GPT-2 block) for Trainium2, 8 NeuronCores.

Sharding: core = 2*batch + head_group. Each of the 8 cores handles one of
B=4 batches and one group of 8 of the 16 heads (Megatron column-split of
the QKV weights, row-split of the proj weights). The two head-group
partial proj outputs per batch are summed on the host; the V-bias and
proj-bias terms are folded into a single host-side additive correction
(softmax rows sum to 1, so attn @ (1 x bv) == bv broadcast).

On-core layout (4-byte float32r matmul operands; PE at 1 cycle/row):
  xT    [128, 8, S]   x transposed via PE transpose-mode (per s-chunk)
  QT/KT [128, 4, S]   feature-major: partition p, slice j <-> feature j*128+p
                      head h lives at partitions (h%2)*64.. , slice h//2
  V     [128, 16, 8, 65]  natural [s, feat] per head + ones column (row sums)
  attnT [128, 4, S]   attention output, feature-major (proj stationary)

Attention per (head, 512-wide q-chunk): scoresT blocks [128 k, <=512 q]
via KT-block.T @ QT (contract 64; a head PAIR occupies array halves 0/64
concurrently via tile_position), additive -1e30 causal mask on the
128-col diagonal corner, exp on ScalarE (1/8 scale folded in), PV +
row-sums via the V ones-column, then a parallel-lane reciprocal
(DMA-scattered to [128,4]) and a DRAM-bounced partition broadcast.

Scheduling: the kernel is emitted as one interleaved stream — attention
chunk q-1 and proj chunk q-2 thunks are dripped between the QKV matmul
groups of chunk q. The dense contract-128 QKV/proj matmuls keep the PE
HAM activity monitor above threshold so the array stays at 2.4 GHz; a
segregated attention phase was measured to fall back to 1.2 GHz.
"""

import os

import numpy as np

import concourse.bass as bass
import concourse.tile as tile
from concourse import bacc, mybir
from concourse.bass_utils import run_bass_kernel_spmd
from concourse.masks import make_identity, make_lower_triangular

# Problem shape (fixed by the harness contract).
B, S, D, H, HD = 4, 2048, 1024, 16, 64
NCORES = 8
HG = 8                # heads per core
FG = HG * HD          # 512 features per head group
P = 128
DB = D // P           # 8 contraction blocks
FBN = FG // P         # 4 feature blocks
SC = 512              # attention sequence chunk
NQ = S // SC          # 4
NKB = S // P          # 16 key blocks
F32 = mybir.dt.float32
F32R = mybir.dt.float32r
BF16 = mybir.dt.bfloat16
DT_MM = BF16 if os.environ.get("KERNEL_DT", "f32r") == "bf16" else F32R
EXP = mybir.ActivationFunctionType.Exp
SCALE = 1.0 / float(HD) ** 0.5
MASKVAL = -1e30


class _Ctx:
    """Tiles/pools shared by the emission thunks."""


def _attention_pair_thunks(nc, cx, hA, hB, q):
    """Thunks emitting one q-chunk (width SC=256) of attention for a pair
    of heads. QT/attnT are per-chunk tiles (cx.QTc[q], cx.attnTc[q])."""
    # Full-width band blocks first (block 0 carries start=True), then the
    # diagonal k-blocks with narrowing width.
    blocks = [(kb, None) for kb in range(4 * q)] + \
             [(4 * q + jj, jj) for jj in range(4)]
    nblk = len(blocks)
    st = {}

    def setup():
        st["heads"] = []
        for h in (hA, hB):
            out_ps = cx.psout.tile([65, SC], F32, tag="outps")
            st["heads"].append((h, (h % 2) * 64, h // 2, out_ps))

    def make_blk(i, kb, jj):
        def run():
            heads = st["heads"]
            off = 0 if jj is None else jj * P
            w = SC - off
            sts = []
            for h, pb, j, out_ps in heads:
                stp = cx.psst.tile([P, SC], F32, tag="stps")
                nc.tensor.matmul(
                    stp[:, :w],
                    cx.KT[pb:pb + 64, j, kb * P:(kb + 1) * P],
                    cx.QTc[q][pb:pb + 64, j, off:SC],
                    start=True, stop=True, tile_position=(pb, 0))
                if jj is not None:
                    nc.vector.tensor_add(stp[:, :P], stp[:, :P], cx.addmask)
                sts.append(stp)
            sxs = []
            for (h, pb, j, out_ps), stp in zip(heads, sts):
                sx = cx.sxp.tile([P, SC], DT_MM, tag="sx")
                nc.scalar.activation(sx[:, :w], stp[:, :w], EXP, scale=SCALE)
                sxs.append(sx)
            for (h, pb, j, out_ps), sx in zip(heads, sxs):
                nc.tensor.matmul(
                    out_ps[:, off:], cx.V[:, kb, h, :], sx[:, :w],
                    start=(i == 0), stop=(i == nblk - 1))
        return run

    def drain():
        st["raws"] = []
        for h, pb, j, out_ps in st["heads"]:
            raw = cx.nrmraw.tile([65, SC], F32, tag="raw")
            nc.vector.tensor_copy(raw, out_ps)
            st["raws"].append(raw)

    def norm():
        for (h, pb, j, out_ps), raw in zip(st["heads"], st["raws"]):
            # Single-partition reciprocal blocks the DVE FIFO for ~us;
            # DMA-scatter the sums across 128 partitions first.
            rsh = cx.nrmbc.tile([P, SC // P], F32, tag="rsh")
            nc.sync.dma_start(rsh, raw[64:65, :])
            nc.vector.reciprocal(rsh, rsh)
            rdram = cx.drp.tile([1, SC], F32, tag="rdram")
            nc.sync.dma_start(rdram, rsh)
            rb = cx.nrmbc.tile([64, SC], F32, tag="rb")
            nc.sync.dma_start(rb, rdram.to_broadcast([64, SC]))
            stg = cx.nrmbc.tile([64, SC], DT_MM, tag="stg")
            nc.vector.tensor_mul(stg, raw[0:64, :], rb)
            nc.sync.dma_start(cx.attnTc[q][pb:pb + 64, j, :], stg)

    thunks = [setup]
    thunks += [make_blk(i, kb, jj) for i, (kb, jj) in enumerate(blocks)]
    thunks += [drain, norm]
    return thunks


def _attention_chunk_thunks(nc, cx, q):
    out = []
    for hp in range(HG // 2):
        out += _attention_pair_thunks(nc, cx, 2 * hp, 2 * hp + 1, q)
    return out


def _proj_chunk_thunks(nc, cx, q, out_d):
    """Proj for the s-blocks of chunk q; two thunks per s-block."""
    thunks = []
    for sb in range(SC // P):
        sblk = q * (SC // P) + sb

        def make_half(hf, sblk=sblk, sb=sb):
            def run():
                og = cx.ogp.tile([P, D // 2], F32, tag="og")
                ps = cx.psst.tile([P, D // 2], F32, tag="stps")
                n0 = hf * (D // 2)
                for j in range(FBN):
                    nc.tensor.matmul(
                        ps,
                        cx.attnTc[q][:, j, sb * P:(sb + 1) * P],
                        cx.wp_sb[:, j, n0:n0 + D // 2],
                        start=(j == 0), stop=(j == FBN - 1))
                nc.any.tensor_copy(og, ps)
                nc.sync.dma_start(
                    out_d.ap()[sblk * P:(sblk + 1) * P, n0:n0 + D // 2], og)
            return run

        thunks.append(make_half(0))
        thunks.append(make_half(1))
    return thunks


def _body(tc, x_d, wq_d, wk_d, wv_d, wp_d, bq_d, bk_d, out_d):
    nc = tc.nc
    cx = _Ctx()
    XC = 256                  # QKV s-chunk width
    NXC = S // XC             # 8
    with (
        tc.tile_pool(name="persist", bufs=1) as persist,
        tc.tile_pool(name="ph1", bufs=1) as ph1,
        tc.tile_pool(name="xin", bufs=3) as xinp,
        tc.tile_pool(name="xtp", bufs=2) as xtp,
        tc.tile_pool(name="qtc", bufs=2) as qtc,
        tc.tile_pool(name="atc", bufs=2) as atc,
        tc.tile_pool(name="sxp", bufs=3) as sxp,
        tc.tile_pool(name="nrmraw", bufs=3) as nrmraw,
        tc.tile_pool(name="nrmbc", bufs=2) as nrmbc,
        tc.tile_pool(name="ogp", bufs=2) as ogp,
        # PSUM banks: qkps 2 + pt 2 + stps (shared st/proj) 2 + outps 2 = 8
        tc.tile_pool(name="ps1", bufs=2, space="PSUM") as ps1,
        tc.tile_pool(name="psst", bufs=2, space="PSUM") as psst,
        tc.tile_pool(name="psout", bufs=2, space="PSUM") as psout,
        tc.tile_pool(name="drp", bufs=8, space="DRAM") as drp,
    ):
        cx.sxp, cx.nrmraw, cx.nrmbc, cx.ogp = sxp, nrmraw, nrmbc, ogp
        cx.psst, cx.psout, cx.drp = psst, psout, drp

        ident = persist.tile([P, P], F32)
        make_identity(nc, ident)
        cx.addmask = persist.tile([P, P], F32)
        make_lower_triangular(nc, cx.addmask, val=MASKVAL, diag=False)
        bq_sb = persist.tile([P, FBN], F32)
        bk_sb = persist.tile([P, FBN], F32)
        nc.sync.dma_start(bq_sb, bq_d.ap().rearrange("(j p) -> p j", p=P))
        nc.sync.dma_start(bk_sb, bk_d.ap().rearrange("(j p) -> p j", p=P))

        cx.KT = persist.tile([P, FBN, S], DT_MM)
        cx.V = persist.tile([P, NKB, HG, HD + 1], DT_MM)
        ones_col = persist.tile([P, 1], F32)
        nc.vector.memset(ones_col, 1.0)
        nc.vector.tensor_copy(cx.V[:, :, :, HD],
                              ones_col.to_broadcast([P, NKB, HG]))
        cx.wp_sb = persist.tile([P, FBN, D], DT_MM)
        nc.sync.dma_start(
            cx.wp_sb, wp_d.ap().rearrange("(j p) n -> p j n", p=P))
        cx.QTc = [qtc.tile([P, FBN, SC], DT_MM, tag="qtc", name=f"qtc{q}")
                  for q in range(NQ)]
        cx.attnTc = [atc.tile([P, FBN, SC], DT_MM, tag="atc",
                              name=f"atc{q}") for q in range(NQ)]

        wq_sb = ph1.tile([P, DB, FG], DT_MM)
        wk_sb = ph1.tile([P, DB, FG], DT_MM)
        wv_sb = ph1.tile([P, DB, FG], DT_MM)
        for w_sb, w_d in ((wq_sb, wq_d), (wk_sb, wk_d), (wv_sb, wv_d)):
            nc.sync.dma_start(
                w_sb, w_d.ap().rearrange("(db p) f -> p db f", p=P))

        def transpose_chunk(xc, xt):
            thunks = []
            for sb in range(XC // P):
                s0 = xc * XC + sb * P
                for dh in range(2):
                    xin = xinp.tile([P, D // 2], F32, tag="xin")
                    nc.sync.dma_start(
                        xin, x_d.ap()[s0:s0 + P,
                                      dh * (D // 2):(dh + 1) * (D // 2)])
                    for db4 in range(DB // 2):
                        db = dh * (DB // 2) + db4
                        def t(sb=sb, db=db, db4=db4, xin=xin):
                            pt = ps1.tile([P, P], F32, tag="pt")
                            nc.tensor.transpose(
                                pt, xin[:, db4 * P:(db4 + 1) * P], ident)
                            nc.any.tensor_copy(
                                xt[:, db, sb * P:(sb + 1) * P], pt)
                        thunks.append(t)
            return thunks

        xts = [xtp.tile([P, DB, XC], DT_MM, tag="xt", name=f"xt{xc}")
               for xc in range(NXC)]

        bg = []          # attention/proj thunks dripped between QKV groups
        tr = []          # transpose thunks for the next chunk

        def drip(ntr, nbg):
            for _ in range(ntr):
                if tr:
                    tr.pop(0)()
            for _ in range(nbg):
                if bg:
                    bg.pop(0)()

        for t in transpose_chunk(0, xts[0]):
            t()

        for xc in range(NXC):
            xt = xts[xc]
            q, half = divmod(xc, 2)
            if xc + 1 < NXC:
                tr += transpose_chunk(xc + 1, xts[xc + 1])
            if half == 0:
                if q >= 1:
                    bg += _attention_chunk_thunks(nc, cx, q - 1)
                if q >= 2:
                    bg += _proj_chunk_thunks(nc, cx, q - 2, out_d)
            per = (len(bg) + 9) // 10

            # Q and K -> transposed feature-major layout, bias added.
            for w_sb, Tc, b_sb in ((wq_sb, cx.QTc, bq_sb),
                                   (wk_sb, None, bk_sb)):
                for fb in range(FBN):
                    ps = ps1.tile([P, XC], F32, tag="qkps")
                    for db in range(DB):
                        nc.tensor.matmul(
                            ps,
                            w_sb[:, db, fb * P:(fb + 1) * P],
                            xt[:, db, :],
                            start=(db == 0), stop=(db == DB - 1))
                        drip(1 if db % 2 else 0, 0)
                    if Tc is not None:
                        dst = Tc[q][:, fb, half * XC:(half + 1) * XC]
                    else:
                        dst = cx.KT[:, fb, xc * XC:(xc + 1) * XC]
                    nc.vector.tensor_scalar_add(dst, ps, b_sb[:, fb:fb + 1])
                    drip(0, per)
            # V -> natural [s, feat] layout (no bias: folded on host).
            for sb in range(XC // P):
                kb = xc * (XC // P) + sb
                ps = ps1.tile([P, FG], F32, tag="qkps")
                for db in range(DB):
                    nc.tensor.matmul(
                        ps,
                        xt[:, db, sb * P:(sb + 1) * P],
                        wv_sb[:, db, :],
                        start=(db == 0), stop=(db == DB - 1))
                    drip(1 if db % 2 else 0, 0)
                nc.vector.tensor_copy(
                    cx.V[:, kb, :, 0:HD],
                    ps.rearrange("p (h c) -> p h c", h=HG))
                drip(0, per)
            while tr:
                tr.pop(0)()

        def junk_mm():
            # Full-array matmul on resident data; result never read. Keeps
            # the PE HAM activity monitor above its warm threshold through
            # the attention tail, which otherwise drops to 1.2 GHz.
            jp = cx.psst.tile([P, SC], F32, tag="stps")
            nc.tensor.matmul(jp, cx.wp_sb[:, 0, 0:P], cx.KT[:, 0, 0:SC],
                             start=True, stop=True)

        # Tail: attention(3) interleaved with proj(2), then proj(3).
        tail_att = _attention_chunk_thunks(nc, cx, NQ - 1)
        tail_proj = _proj_chunk_thunks(nc, cx, NQ - 2, out_d)
        k = max(1, len(tail_att) // max(1, len(tail_proj)))
        nt = 0
        while tail_att or tail_proj:
            for _ in range(k):
                if tail_att:
                    tail_att.pop(0)()
                    nt += 1
                    if nt % 2 == 0:
                        junk_mm()
            if tail_proj:
                tail_proj.pop(0)()
        while bg:
            bg.pop(0)()
        for t in _proj_chunk_thunks(nc, cx, NQ - 1, out_d):
            t()


def build_nc():
    nc = bacc.Bacc("TRN2", target_bir_lowering=False)
    x_d = nc.dram_tensor("x", [S, D], F32, kind="ExternalInput")
    wq_d = nc.dram_tensor("wq", [D, FG], DT_MM, kind="ExternalInput")
    wk_d = nc.dram_tensor("wk", [D, FG], DT_MM, kind="ExternalInput")
    wv_d = nc.dram_tensor("wv", [D, FG], DT_MM, kind="ExternalInput")
    wp_d = nc.dram_tensor("wp", [FG, D], DT_MM, kind="ExternalInput")
    bq_d = nc.dram_tensor("bq", [FG], F32, kind="ExternalInput")
    bk_d = nc.dram_tensor("bk", [FG], F32, kind="ExternalInput")
    out_d = nc.dram_tensor("out", [S, D], F32, kind="ExternalOutput")
    with tile.TileContext(nc) as tc:
        _body(tc, x_d, wq_d, wk_d, wv_d, wp_d, bq_d, bk_d, out_d)
    nc.compile()
    return nc


_NC = None


def _get_nc():
    global _NC
    if _NC is None:
        _NC = build_nc()
    return _NC


def make_in_maps(hs, w, bvec, pw):
    import ml_dtypes
    wdt = ml_dtypes.bfloat16 if DT_MM == BF16 else np.float32
    in_maps = []
    for core in range(NCORES):
        b, g = divmod(core, 2)
        lo, hi = g * FG, (g + 1) * FG
        in_maps.append({
            "x": np.ascontiguousarray(hs[b]),
            "wq": np.ascontiguousarray(w[:, lo:hi]).astype(wdt),
            "wk": np.ascontiguousarray(w[:, D + lo:D + hi]).astype(wdt),
            "wv": np.ascontiguousarray(
                w[:, 2 * D + lo:2 * D + hi]).astype(wdt),
            "wp": np.ascontiguousarray(pw[lo:hi, :]).astype(wdt),
            "bq": np.ascontiguousarray(bvec[lo:hi]),
            "bk": np.ascontiguousarray(bvec[D + lo:D + hi]),
        })
    return in_maps


def combine(parts, bvec, pw, pb):
    bv = bvec[2 * D:3 * D].astype(np.float64)
    corr = (bv @ pw.astype(np.float64) + pb.astype(np.float64)).astype(
        np.float32)
    out = np.empty((B, S, D), np.float32)
    for b in range(B):
        out[b] = parts[2 * b] + parts[2 * b + 1] + corr
    return out


def kernel(hidden_states, c_attn_w, c_attn_b, c_proj_w, c_proj_b,
           **run_kwargs):
    hs = np.asarray(hidden_states, dtype=np.float32)
    w = np.asarray(c_attn_w, dtype=np.float32)
    bvec = np.asarray(c_attn_b, dtype=np.float32)
    pw = np.asarray(c_proj_w, dtype=np.float32)
    pb = np.asarray(c_proj_b, dtype=np.float32)
    nc = _get_nc()
    res = run_bass_kernel_spmd(nc, make_in_maps(hs, w, bvec, pw),
                               core_ids=list(range(NCORES)), **run_kwargs)
    parts = [res.results[i]["out"] for i in range(NCORES)]
    out = combine(parts, bvec, pw, pb)
    if run_kwargs:
        return out, res
    return out


# revision 27
# speedup vs baseline: 1.1526x; 1.1526x over previous
"""Causal self-attention (GPT-2 block) for Trainium2, 8 NeuronCores.

Sharding: core = 2*batch + head_group. Each of the 8 cores handles one of
B=4 batches and one group of 8 of the 16 heads (Megatron column-split of
the QKV weights, row-split of the proj weights). The two head-group
partial proj outputs per batch are summed on the host; the V-bias and
proj-bias terms are folded into a single host-side additive correction
(softmax rows sum to 1, so attn @ (1 x bv) == bv broadcast).

On-core layout (4-byte float32r matmul operands; PE at 1 cycle/row):
  xT    [128, 8, S]   x transposed via PE transpose-mode (per s-chunk)
  QT/KT [128, 4, S]   feature-major: partition p, slice j <-> feature j*128+p
                      head h lives at partitions (h%2)*64.. , slice h//2
  V     [128, 16, 8, 65]  natural [s, feat] per head + ones column (row sums)
  attnT [128, 4, S]   attention output, feature-major (proj stationary)

Attention per (head, 512-wide q-chunk): scoresT blocks [128 k, <=512 q]
via KT-block.T @ QT (contract 64; a head PAIR occupies array halves 0/64
concurrently via tile_position), additive -1e30 causal mask on the
128-col diagonal corner, exp on ScalarE (1/8 scale folded in), PV +
row-sums via the V ones-column, then a parallel-lane reciprocal
(DMA-scattered to [128,4]) and a DRAM-bounced partition broadcast.

Scheduling: the kernel is emitted as one interleaved stream — attention
chunk q-1 and proj chunk q-2 thunks are dripped between the QKV matmul
groups of chunk q. The dense contract-128 QKV/proj matmuls keep the PE
HAM activity monitor above threshold so the array stays at 2.4 GHz; a
segregated attention phase was measured to fall back to 1.2 GHz.
"""

import os

import numpy as np

import concourse.bass as bass
import concourse.tile as tile
from concourse import bacc, mybir
from concourse.bass_utils import run_bass_kernel_spmd
from concourse.masks import make_identity, make_lower_triangular

# Problem shape (fixed by the harness contract).
B, S, D, H, HD = 4, 2048, 1024, 16, 64
NCORES = 8
HG = 8                # heads per core
FG = HG * HD          # 512 features per head group
P = 128
DB = D // P           # 8 contraction blocks
FBN = FG // P         # 4 feature blocks
SC = 512              # attention sequence chunk
NQ = S // SC          # 4
NKB = S // P          # 16 key blocks
F32 = mybir.dt.float32
F32R = mybir.dt.float32r
BF16 = mybir.dt.bfloat16
DT_MM = BF16 if os.environ.get("KERNEL_DT", "f32r") == "bf16" else F32R
EXP = mybir.ActivationFunctionType.Exp
SCALE = 1.0 / float(HD) ** 0.5
MASKVAL = -1e30


class _Ctx:
    """Tiles/pools shared by the emission thunks."""


def _attention_pair_thunks(nc, cx, hA, hB, q):
    """Thunks emitting one q-chunk (width SC=256) of attention for a pair
    of heads. QT/attnT are per-chunk tiles (cx.QTc[q], cx.attnTc[q])."""
    # Full-width band blocks first (block 0 carries start=True), then the
    # diagonal k-blocks with narrowing width.
    blocks = [(kb, None) for kb in range(4 * q)] + \
             [(4 * q + jj, jj) for jj in range(4)]
    nblk = len(blocks)
    st = {}

    def setup():
        st["heads"] = []
        for h in (hA, hB):
            out_ps = cx.psout.tile([65, SC], F32, tag="outps")
            st["heads"].append((h, (h % 2) * 64, h // 2, out_ps))

    def make_blk(i, kb, jj):
        def run():
            heads = st["heads"]
            off = 0 if jj is None else jj * P
            w = SC - off
            sts = []
            for h, pb, j, out_ps in heads:
                stp = cx.psst.tile([P, SC], F32, tag="stps")
                nc.tensor.matmul(
                    stp[:, :w],
                    cx.KT[pb:pb + 64, j, kb * P:(kb + 1) * P],
                    cx.QTc[q][pb:pb + 64, j, off:SC],
                    start=True, stop=True, tile_position=(pb, 0))
                if jj is not None:
                    nc.vector.tensor_add(stp[:, :P], stp[:, :P], cx.addmask)
                sts.append(stp)
            sxs = []
            for (h, pb, j, out_ps), stp in zip(heads, sts):
                sx = cx.sxp.tile([P, SC], DT_MM, tag="sx")
                nc.scalar.activation(sx[:, :w], stp[:, :w], EXP, scale=SCALE)
                sxs.append(sx)
            for (h, pb, j, out_ps), sx in zip(heads, sxs):
                nc.tensor.matmul(
                    out_ps[:, off:], cx.V[:, kb, h, :], sx[:, :w],
                    start=(i == 0), stop=(i == nblk - 1))
        return run

    def drain():
        st["raws"] = []
        for h, pb, j, out_ps in st["heads"]:
            raw = cx.nrmraw.tile([65, SC], F32, tag="raw")
            nc.vector.tensor_copy(raw, out_ps)
            st["raws"].append(raw)

    def norm():
        for (h, pb, j, out_ps), raw in zip(st["heads"], st["raws"]):
            # Single-partition reciprocal blocks the DVE FIFO for ~us;
            # DMA-scatter the sums across 128 partitions first.
            rsh = cx.nrmbc.tile([P, SC // P], F32, tag="rsh")
            nc.sync.dma_start(rsh, raw[64:65, :])
            nc.vector.reciprocal(rsh, rsh)
            rdram = cx.drp.tile([1, SC], F32, tag="rdram")
            nc.sync.dma_start(rdram, rsh)
            rb = cx.nrmbc.tile([64, SC], F32, tag="rb")
            nc.sync.dma_start(rb, rdram.to_broadcast([64, SC]))
            stg = cx.nrmbc.tile([64, SC], DT_MM, tag="stg")
            nc.vector.tensor_mul(stg, raw[0:64, :], rb)
            nc.sync.dma_start(cx.attnTc[q][pb:pb + 64, j, :], stg)

    thunks = [setup]
    thunks += [make_blk(i, kb, jj) for i, (kb, jj) in enumerate(blocks)]
    thunks += [drain, norm]
    return thunks


def _attention_chunk_thunks(nc, cx, q):
    out = []
    for hp in range(HG // 2):
        out += _attention_pair_thunks(nc, cx, 2 * hp, 2 * hp + 1, q)
    return out


def _proj_chunk_thunks(nc, cx, q, out_d):
    """Proj for the s-blocks of chunk q; two thunks per s-block."""
    thunks = []
    for sb in range(SC // P):
        sblk = q * (SC // P) + sb

        def make_half(hf, sblk=sblk, sb=sb):
            def run():
                og = cx.ogp.tile([P, D // 2], F32, tag="og")
                ps = cx.ps1.tile([P, D // 2], F32, tag="qkps")
                n0 = hf * (D // 2)
                for j in range(FBN):
                    nc.tensor.matmul(
                        ps,
                        cx.attnTc[q][:, j, sb * P:(sb + 1) * P],
                        cx.wp_sb[:, j, n0:n0 + D // 2],
                        start=(j == 0), stop=(j == FBN - 1))
                nc.any.tensor_copy(og, ps)
                nc.sync.dma_start(
                    out_d.ap()[sblk * P:(sblk + 1) * P, n0:n0 + D // 2], og)
            return run

        thunks.append(make_half(0))
        thunks.append(make_half(1))
    return thunks


def _body(tc, x_d, wq_d, wk_d, wv_d, wp_d, bq_d, bk_d, out_d):
    nc = tc.nc
    cx = _Ctx()
    XC = 256                  # QKV s-chunk width
    NXC = S // XC             # 8
    with (
        tc.tile_pool(name="persist", bufs=1) as persist,
        tc.tile_pool(name="ph1", bufs=1) as ph1,
        tc.tile_pool(name="xin", bufs=3) as xinp,
        tc.tile_pool(name="xtp", bufs=2) as xtp,
        tc.tile_pool(name="qtc", bufs=2) as qtc,
        tc.tile_pool(name="atc", bufs=2) as atc,
        tc.tile_pool(name="sxp", bufs=3) as sxp,
        tc.tile_pool(name="nrmraw", bufs=3) as nrmraw,
        tc.tile_pool(name="nrmbc", bufs=2) as nrmbc,
        tc.tile_pool(name="ogp", bufs=2) as ogp,
        # PSUM banks: qkps 2 + pt 2 + stps (shared st/proj) 2 + outps 2 = 8
        tc.tile_pool(name="ps1", bufs=2, space="PSUM") as ps1,
        tc.tile_pool(name="psst", bufs=2, space="PSUM") as psst,
        tc.tile_pool(name="psout", bufs=2, space="PSUM") as psout,
        tc.tile_pool(name="drp", bufs=8, space="DRAM") as drp,
    ):
        cx.sxp, cx.nrmraw, cx.nrmbc, cx.ogp = sxp, nrmraw, nrmbc, ogp
        cx.psst, cx.psout, cx.drp, cx.ps1 = psst, psout, drp, ps1

        ident = persist.tile([P, P], F32)
        make_identity(nc, ident)
        for _ in range(12):
            wp_ps = ps1.tile([P, P], F32, tag="pt")
            nc.tensor.matmul(wp_ps, ident, ident, start=True, stop=True)
        cx.addmask = persist.tile([P, P], F32)
        make_lower_triangular(nc, cx.addmask, val=MASKVAL, diag=False)
        bq_sb = persist.tile([P, FBN], F32)
        bk_sb = persist.tile([P, FBN], F32)
        nc.sync.dma_start(bq_sb, bq_d.ap().rearrange("(j p) -> p j", p=P))
        nc.sync.dma_start(bk_sb, bk_d.ap().rearrange("(j p) -> p j", p=P))

        cx.KT = persist.tile([P, FBN, S], DT_MM)
        cx.V = persist.tile([P, NKB, HG, HD + 1], DT_MM)
        ones_col = persist.tile([P, 1], F32)
        nc.vector.memset(ones_col, 1.0)
        nc.vector.tensor_copy(cx.V[:, :, :, HD],
                              ones_col.to_broadcast([P, NKB, HG]))
        cx.wp_sb = persist.tile([P, FBN, D], DT_MM)
        nc.sync.dma_start(
            cx.wp_sb, wp_d.ap().rearrange("(j p) n -> p j n", p=P))
        cx.QTc = [qtc.tile([P, FBN, SC], DT_MM, tag="qtc", name=f"qtc{q}")
                  for q in range(NQ)]
        cx.attnTc = [atc.tile([P, FBN, SC], DT_MM, tag="atc",
                              name=f"atc{q}") for q in range(NQ)]

        wq_sb = ph1.tile([P, DB, FG], DT_MM)
        wk_sb = ph1.tile([P, DB, FG], DT_MM)
        wv_sb = ph1.tile([P, DB, FG], DT_MM)
        for w_sb, w_d in ((wq_sb, wq_d), (wk_sb, wk_d), (wv_sb, wv_d)):
            nc.sync.dma_start(
                w_sb, w_d.ap().rearrange("(db p) f -> p db f", p=P))

        def transpose_chunk(xc, xt):
            thunks = []
            for sb in range(XC // P):
                s0 = xc * XC + sb * P
                for dh in range(2):
                    xin = xinp.tile([P, D // 2], F32, tag="xin")
                    nc.sync.dma_start(
                        xin, x_d.ap()[s0:s0 + P,
                                      dh * (D // 2):(dh + 1) * (D // 2)])
                    for db4 in range(DB // 2):
                        db = dh * (DB // 2) + db4
                        def t(sb=sb, db=db, db4=db4, xin=xin):
                            pt = ps1.tile([P, P], F32, tag="pt")
                            nc.tensor.transpose(
                                pt, xin[:, db4 * P:(db4 + 1) * P], ident)
                            nc.any.tensor_copy(
                                xt[:, db, sb * P:(sb + 1) * P], pt)
                        thunks.append(t)
            return thunks

        xts = [xtp.tile([P, DB, XC], DT_MM, tag="xt", name=f"xt{xc}")
               for xc in range(NXC)]

        bg = []          # attention/proj thunks dripped between QKV groups
        tr = []          # transpose thunks for the next chunk

        def drip(ntr, nbg):
            for _ in range(ntr):
                if tr:
                    tr.pop(0)()
            for _ in range(nbg):
                if bg:
                    bg.pop(0)()

        for t in transpose_chunk(0, xts[0]):
            t()

        for xc in range(NXC):
            xt = xts[xc]
            q, half = divmod(xc, 2)
            if xc + 1 < NXC:
                tr += transpose_chunk(xc + 1, xts[xc + 1])
            if half == 0:
                if q >= 1:
                    bg += _attention_chunk_thunks(nc, cx, q - 1)
                if q >= 2:
                    bg += _proj_chunk_thunks(nc, cx, q - 2, out_d)
            per = (len(bg) + 9) // 10

            # Q and K -> transposed feature-major layout, bias added.
            for w_sb, Tc, b_sb in ((wq_sb, cx.QTc, bq_sb),
                                   (wk_sb, None, bk_sb)):
                for fb in range(FBN):
                    ps = ps1.tile([P, XC], F32, tag="qkps")
                    for db in range(DB):
                        nc.tensor.matmul(
                            ps,
                            w_sb[:, db, fb * P:(fb + 1) * P],
                            xt[:, db, :],
                            start=(db == 0), stop=(db == DB - 1))
                        drip(1 if db % 2 else 0, 0)
                    if Tc is not None:
                        dst = Tc[q][:, fb, half * XC:(half + 1) * XC]
                    else:
                        dst = cx.KT[:, fb, xc * XC:(xc + 1) * XC]
                    nc.vector.tensor_scalar_add(dst, ps, b_sb[:, fb:fb + 1])
                    drip(0, per)
            # V -> natural [s, feat] layout (no bias: folded on host).
            for sb in range(XC // P):
                kb = xc * (XC // P) + sb
                ps = ps1.tile([P, FG], F32, tag="qkps")
                for db in range(DB):
                    nc.tensor.matmul(
                        ps,
                        xt[:, db, sb * P:(sb + 1) * P],
                        wv_sb[:, db, :],
                        start=(db == 0), stop=(db == DB - 1))
                    drip(1 if db % 2 else 0, 0)
                nc.vector.tensor_copy(
                    cx.V[:, kb, :, 0:HD],
                    ps.rearrange("p (h c) -> p h c", h=HG))
                drip(0, per)
            while tr:
                tr.pop(0)()

        # Tail: attention(3) interleaved with proj(2), then proj(3).
        tail_att = _attention_chunk_thunks(nc, cx, NQ - 1)
        tail_proj = _proj_chunk_thunks(nc, cx, NQ - 2, out_d)
        k = max(1, len(tail_att) // max(1, len(tail_proj)))
        while tail_att or tail_proj:
            for _ in range(k):
                if tail_att:
                    tail_att.pop(0)()
            if tail_proj:
                tail_proj.pop(0)()
        while bg:
            bg.pop(0)()
        for t in _proj_chunk_thunks(nc, cx, NQ - 1, out_d):
            t()


def build_nc():
    nc = bacc.Bacc("TRN2", target_bir_lowering=False)
    x_d = nc.dram_tensor("x", [S, D], F32, kind="ExternalInput")
    wq_d = nc.dram_tensor("wq", [D, FG], DT_MM, kind="ExternalInput")
    wk_d = nc.dram_tensor("wk", [D, FG], DT_MM, kind="ExternalInput")
    wv_d = nc.dram_tensor("wv", [D, FG], DT_MM, kind="ExternalInput")
    wp_d = nc.dram_tensor("wp", [FG, D], DT_MM, kind="ExternalInput")
    bq_d = nc.dram_tensor("bq", [FG], F32, kind="ExternalInput")
    bk_d = nc.dram_tensor("bk", [FG], F32, kind="ExternalInput")
    out_d = nc.dram_tensor("out", [S, D], F32, kind="ExternalOutput")
    with tile.TileContext(nc) as tc:
        _body(tc, x_d, wq_d, wk_d, wv_d, wp_d, bq_d, bk_d, out_d)
    nc.compile()
    return nc


_NC = None


def _get_nc():
    global _NC
    if _NC is None:
        _NC = build_nc()
    return _NC


def make_in_maps(hs, w, bvec, pw):
    import ml_dtypes
    wdt = ml_dtypes.bfloat16 if DT_MM == BF16 else np.float32
    in_maps = []
    for core in range(NCORES):
        b, g = divmod(core, 2)
        lo, hi = g * FG, (g + 1) * FG
        in_maps.append({
            "x": np.ascontiguousarray(hs[b]),
            "wq": np.ascontiguousarray(w[:, lo:hi]).astype(wdt),
            "wk": np.ascontiguousarray(w[:, D + lo:D + hi]).astype(wdt),
            "wv": np.ascontiguousarray(
                w[:, 2 * D + lo:2 * D + hi]).astype(wdt),
            "wp": np.ascontiguousarray(pw[lo:hi, :]).astype(wdt),
            "bq": np.ascontiguousarray(bvec[lo:hi]),
            "bk": np.ascontiguousarray(bvec[D + lo:D + hi]),
        })
    return in_maps


def combine(parts, bvec, pw, pb):
    bv = bvec[2 * D:3 * D].astype(np.float64)
    corr = (bv @ pw.astype(np.float64) + pb.astype(np.float64)).astype(
        np.float32)
    out = np.empty((B, S, D), np.float32)
    for b in range(B):
        out[b] = parts[2 * b] + parts[2 * b + 1] + corr
    return out


def kernel(hidden_states, c_attn_w, c_attn_b, c_proj_w, c_proj_b,
           **run_kwargs):
    hs = np.asarray(hidden_states, dtype=np.float32)
    w = np.asarray(c_attn_w, dtype=np.float32)
    bvec = np.asarray(c_attn_b, dtype=np.float32)
    pw = np.asarray(c_proj_w, dtype=np.float32)
    pb = np.asarray(c_proj_b, dtype=np.float32)
    nc = _get_nc()
    res = run_bass_kernel_spmd(nc, make_in_maps(hs, w, bvec, pw),
                               core_ids=list(range(NCORES)), **run_kwargs)
    parts = [res.results[i]["out"] for i in range(NCORES)]
    out = combine(parts, bvec, pw, pb)
    if run_kwargs:
        return out, res
    return out
